# revision 31
# baseline (speedup 1.0000x reference)
"""Self-contained TRN2 Bass kernel for nn_EuclideanSimilarity.

Full-input contract: kernel(x, W, b) with
  x [4, 4096, 128] f32, W [128, 128] f32, b [128] f32
returns out [4, 4096, 4096] f32 = exp(-pairwise_euclidean_dist(x @ W.T + b)).

Sharding + symmetry: 8 cores, core c -> (batch c//2, half c%2).  Each batch's
similarity matrix is SYMMETRIC, so a core computes only the chunk-level upper
triangle of its two 2048x2048 quadrants: self (own-half queries x own-half
keys) and cross (own-half queries x other-half keys).  Core 2b covers D1 and
B-upper; core 2b+1 covers D2 and (B^T)-upper = B-lower.  The host mirrors the
missing block-lower tiles from their (always directly computed) transposed
counterparts.  Each core's key tensor is ordered [own half | other half], so
all 8 cores run an IDENTICAL program (SPMD); only the host-side column
mapping differs.  Per-core work: 80 of 128 [128,512] output chunks.

Numerics: x^T and W^T are prepared on the host (input marshalling) so the
device does no input transposes.  h^T = W @ x^T + b is computed per 512-token
chunk in fp32 and rounded ONCE to fp32r; queries are the self-half keys, so 8
projection matmuls cover everything.  The gram matmul of fp32r operands is
exact (24-bit products, fp32 accumulate) and the norms are computed from the
SAME rounded values, so d2 = sq[m] + sq[n] - 2 h_m.h_n is the exact pairwise
distance of the rounded vectors - no catastrophic cancellation; h_r vs h is a
~2^-12 relative perturbation, invisible through exp(-sqrt).  The exact
diagonal (true d2 = 0, computed 0 +/- psum accumulation noise, possibly NaN
after sqrt of a tiny negative) is pinned to exp(0) = 1 on the host during
unsharding, which also clears those NaNs.

Per [128,512] chunk the PE does TWO fp32r matmuls: gram (h queries x keys,
K=128) and a K=1 "aug" adding -sq_k[n]/2 (ones lhsT x norm-row rhs).  The
drain then computes d2 = -2*(psum) + sq_q[m]: for ACT-assigned qtiles as one
fused ACT op sqrt(-2*psum + sq_q) straight from PSUM (sqrt table), otherwise
as a DVE tensor_scalar into fp16 followed by sqrt as pow(x, 0.5) on the
GPSIMD (a third elementwise lane).  exp(-x) runs on ACT into bf16 (one exp
table load), leaves as bf16 (half DMA bytes, one packed DMA per qtile), and
is upcast on the host during unsharding.  Qtiles are processed 15->0 (small
ones first - they need only the first-loaded key chunks, so the main loop
starts ~5us in, overlapping the rest of the projection prologue).
"""

from contextlib import ExitStack

import numpy as np

import concourse.mybir as mybir
import concourse.tile as tile
from concourse.tile import add_dep_helper
from concourse import bacc
from concourse.bass import ts
from concourse.masks import make_identity

F32 = mybir.dt.float32
F32R = mybir.dt.float32r
F16 = mybir.dt.float16
BF16 = mybir.dt.bfloat16
AF = mybir.ActivationFunctionType
ALU = mybir.AluOpType

B = 4
N = 4096
NH = 2048   # tokens per half (queries per core)
D = 128
TEMPERATURE = 1.0
NQT = NH // 128  # 16 query tiles per core
N_CORES = 8

# qtiles whose drain+sqrt runs fused on ACT (sqrt table); the rest drain via
# DVE and take sqrt on the GPSIMD pow lane.  The small qtiles run first (they
# need only the first-loaded key chunks, overlapping the prologue), and one
# mid-size ACT qtile sits in the final group to balance the engines.
ACT_SQRT_QT = (15, 14, 13, 12, 7)


def kernel_body(ctx: ExitStack, tc: tile.TileContext, out, xt, wt, b):
    nc = tc.nc

    consts = ctx.enter_context(tc.tile_pool(name="consts", bufs=1))
    # preload the sqrt table set while the prologue runs
    scrap = consts.tile([1, 8], F32)
    nc.gpsimd.memset(scrap[:], 1.0)
    nc.scalar.activation(scrap[:], scrap[:], AF.Sqrt)

    ident = consts.tile([128, 128], F32)
    make_identity(nc, ident[:])

    wt_sb = consts.tile([128, 128], F32)
    nc.sync.dma_start(wt_sb[:], wt[:, :])
    b_sb = consts.tile([128, 1], F32)
    nc.sync.dma_start(b_sb[:], b[:, :])
    ones_col = consts.tile([128, 1], F32)    # lhsT for the f32 sq matmul
    nc.gpsimd.memset(ones_col[:], 1.0)
    ones_row = consts.tile([1, 128], F32)
    nc.gpsimd.memset(ones_row[:], 1.0)
    ones1r = consts.tile([1, 128], F32R)     # K=1 lhsT for the aug matmul
    nc.vector.tensor_copy(ones1r[:], ones_row[:])
    half16 = consts.tile([128, N], F16)      # pow exponent for gpsimd sqrt
    nc.vector.memset(half16[:], 0.5)

    # persistent operands
    h_pool = ctx.enter_context(tc.tile_pool(name="h", bufs=1))
    hk = h_pool.tile([128, N], F32R)         # h for all 4096 keys
    augk = h_pool.tile([1, N], F32R)         # -sq_k/2 row
    sqq_cols = h_pool.tile([128, NQT], F32)  # sq_q column per qtile
    sq_row = h_pool.tile([1, NH], F32)       # self-half norms (fp32)

    # projection pools (coexist with the main loop; 3 PSUM banks)
    ssb = ctx.enter_context(tc.tile_pool(name="setup_sb", bufs=3))
    sps = ctx.enter_context(tc.tile_pool(name="setup_ps", bufs=1, space="PSUM"))

    # PE p-state warmup: ~60 tiny matmuls keep the tensor engine busy from
    # t~1us so its clock is fully ramped (3us of continuous execution) by the
    # time the first real fp32 projection matmul dispatches — those would
    # otherwise run 2-3x slower and serialize the whole prologue.
    warm = sps.tile([128, 16], F32, tag="sqqtp", bufs=1, name="warm")
    for i in range(60):
        nc.tensor.matmul(warm[:], ident[:], ident[:, 0:16],
                         start=True, stop=True)

    # prefetch all key chunks up front (the per-chunk DMA latency otherwise
    # serializes into the projection chain)
    xins = {}
    for c in (3, 7, 2, 6, 1, 5, 0, 4):
        xin = ssb.tile([128, 512], F32, tag="xin", bufs=8, name=f"xin{c}")
        nc.sync.dma_start(xin[:], xt[:, ts(c, 512)])
        xins[c] = xin

    s2fs = {}

    def project_mm(c):
        hps = sps.tile([128, 512], F32, tag="hps", bufs=2, name=f"hps{c}")
        nc.tensor.matmul(hps[:], wt_sb[:], xins[c][:], start=True, stop=True)
        nc.vector.tensor_scalar_add(hk[:, ts(c, 512)], hps[:], b_sb[:, 0:1])
        s2f = ssb.tile([128, 512], F32, tag="s2f", bufs=8, name=f"s2f{c}")
        nc.vector.tensor_mul(s2f[:], hk[:, ts(c, 512)], hk[:, ts(c, 512)])
        s2fs[c] = s2f

    def project_norms(c):
        sqps = sps.tile([128, 512], F32, tag="sqps", bufs=1, name=f"sqps{c}")
        nc.tensor.matmul(
            sqps[0:1, :], ones_col[:], s2fs.pop(c)[:], start=True, stop=True
        )
        nc.scalar.activation(
            augk[0:1, ts(c, 512)], sqps[0:1, :], AF.Identity, scale=-0.5
        )
        if c < NH // 512:  # self chunk: also query norms (exact fp32)
            nc.vector.tensor_copy(sq_row[0:1, ts(c, 512)], sqps[0:1, :])
            for j in range(4):
                qt = 4 * c + j
                nc.tensor.transpose(
                    warm[:, qt % 4:qt % 4 + 1], sq_row[0:1, ts(qt, 128)],
                    ident[0:1, 0:1],
                )
            nc.vector.tensor_copy(
                sqq_cols[:, 4 * c:4 * c + 4], warm[:, 0:4]
            )

    def project_chunk(c):
        project_mm(c)
        project_norms(c)

    # ---------------- main loop ----------------
    st_pool = ctx.enter_context(tc.tile_pool(name="st", bufs=6))
    st2_pool = ctx.enter_context(tc.tile_pool(name="st2", bufs=6))
    ps_pool = ctx.enter_context(tc.tile_pool(name="d2", bufs=2, space="PSUM"))

    last_act = [None]

    def chained_act(*args, **kwargs):
        bi = nc.scalar.activation(*args, **kwargs)
        if last_act[0] is not None:
            add_dep_helper(bi.ins, last_act[0].ins, sync=False,
                           reason="act-table-order")
        last_act[0] = bi
        return bi

    pending = []

    def emit_tail(qt, st, nch, split):
        st2 = st2_pool.tile([128, nch * 512], BF16, tag="st2", name=f"st2_{qt}")
        # NaNs from sqrt of tiny diagonal negatives pass through; the host
        # pins the exact diagonal to exp(0)=1 afterwards.
        if split:  # final qtile: halve exp+DMA so the last DMA overlaps
            h = nch * 256
            chained_act(st2[:, 0:h], st[:, 0:h], AF.Exp, scale=-TEMPERATURE)
            nc.sync.dma_start(out[ts(qt, 128), 0:h], st2[:, 0:h])
            chained_act(st2[:, h:2 * h], st[:, h:2 * h], AF.Exp,
                        scale=-TEMPERATURE)
            nc.sync.dma_start(out[ts(qt, 128), h:2 * h], st2[:, h:2 * h])
        else:
            chained_act(st2[:], st[:], AF.Exp, scale=-TEMPERATURE)
            nc.sync.dma_start(out[ts(qt, 128), 0:nch * 512], st2[:])

    def emit_qtile(qt, last=False):
        cl0 = qt // 4
        nch = 2 * (4 - cl0)
        cols = [cl * 512 for cl in range(cl0, 4)] + \
               [NH + cl * 512 for cl in range(cl0, 4)]
        st = st_pool.tile([128, nch * 512], F16, tag="st", name=f"st{qt}")
        for s0 in range(0, nch, 2):
            seg = cols[s0:s0 + 2]
            ps = ps_pool.tile([128, 1024], F32, tag="d2", name=f"d2_{qt}_{s0}")
            for j, col in enumerate(seg):
                psl = ps[:, ts(j, 512)]
                nc.tensor.matmul(
                    psl, hk[:, ts(qt, 128)], hk[:, col:col + 512],
                    start=True, stop=False,
                )
                nc.tensor.matmul(
                    psl, ones1r[:], augk[:, col:col + 512],
                    start=False, stop=True,
                )
            sw = len(seg) * 512
            sl = st[:, s0 * 512:s0 * 512 + sw]
            if qt in ACT_SQRT_QT:
                # fused drain+sqrt: sqrt(-2*psum + sq_q)
                chained_act(sl, ps[:, 0:sw], AF.Sqrt,
                            bias=sqq_cols[:, qt:qt + 1], scale=-2.0)
            else:
                # d2 = -2*psum + sq_q
                nc.vector.tensor_scalar(
                    sl, ps[:, 0:sw], -2.0, sqq_cols[:, qt:qt + 1],
                    ALU.mult, ALU.add,
                )
                if last or nch == 8:  # seg-granular sqrt: shorter chain
                    nc.gpsimd.tensor_tensor(
                        sl, sl, half16[:, 0:sw], ALU.pow
                    )
        if qt not in ACT_SQRT_QT and not last and nch != 8:
            nc.gpsimd.tensor_tensor(
                st[:], st[:], half16[:, 0:nch * 512], ALU.pow
            )
        pending.append((qt, st, nch, last))
        if qt == 12:  # ACT sqrt-phase over; exps flow from here (1 exp load)
            for item in pending:
                emit_tail(*item)
            pending.clear()
        elif qt < 12:
            emit_tail(*pending.pop())

    # project chunks 3,7 first (the small qtiles need only those), then the
    # ACT-drained small qtiles stream while the remaining chunks project on
    # the warmed PE; after that the whole schedule is projection-free
    project_mm(3)
    project_mm(7)
    project_norms(3)
    project_norms(7)
    for qt in (15, 14, 13, 12):
        emit_qtile(qt)
    project_mm(2)
    project_mm(6)
    project_norms(2)
    project_norms(6)
    for qt in (11, 10, 9, 8):
        emit_qtile(qt)
    project_mm(1)
    project_mm(5)
    project_norms(1)
    project_norms(5)
    project_mm(0)
    project_mm(4)
    project_norms(0)
    project_norms(4)
    # qt7 (the ACT-drained one) first in the tail group so its PSUM tiles
    # recycle early instead of queueing behind the exp chain
    for qt in (7, 3, 2, 1, 0, 6, 5):
        emit_qtile(qt)
    emit_qtile(4, last=True)


def build_nc():
    nc = bacc.Bacc("TRN2", target_bir_lowering=False, debug=False)
    xt = nc.dram_tensor("xt", [D, N], F32, kind="ExternalInput").ap()
    wt = nc.dram_tensor("wt", [D, D], F32, kind="ExternalInput").ap()
    b = nc.dram_tensor("b", [D, 1], F32, kind="ExternalInput").ap()
    out = nc.dram_tensor("out", [NH, N], BF16, kind="ExternalOutput").ap()
    with tile.TileContext(nc) as tc:
        with ExitStack() as ctx:
            kernel_body(ctx, tc, out, xt, wt, b)
    nc.compile()
    return nc


_NC_CACHE = None


def _get_nc():
    global _NC_CACHE
    if _NC_CACHE is None:
        _NC_CACHE = build_nc()
    return _NC_CACHE


def _run(x, W, b, trace=False, **spmd_kwargs):
    from concourse.bass_utils import run_bass_kernel_spmd

    x = np.asarray(x, dtype=np.float32)
    W = np.asarray(W, dtype=np.float32)
    b = np.asarray(b, dtype=np.float32).reshape(D, 1)
    wt = np.ascontiguousarray(W.T)
    nc = _get_nc()
    in_maps = []
    for c in range(N_CORES):
        bi, qh = c // 2, c % 2
        xtb = x[bi].T  # [128, 4096]
        own = xtb[:, qh * NH:(qh + 1) * NH]
        oth = xtb[:, (1 - qh) * NH:(2 - qh) * NH]
        in_maps.append({
            "xt": np.ascontiguousarray(np.concatenate([own, oth], axis=1)),
            "wt": wt,
            "b": b,
        })
    res = run_bass_kernel_spmd(
        nc, in_maps, core_ids=list(range(N_CORES)), trace=trace, **spmd_kwargs
    )
    out = np.empty((B, N, N), dtype=np.float32)
    for c in range(N_CORES):
        bi, qh = c // 2, c % 2
        R = res.results[c]["out"].astype(np.float32)  # [2048, 4096] packed
        rows = slice(qh * NH, (qh + 1) * NH)
        M = out[bi]
        for qt in range(NQT):
            cl0 = qt // 4
            w = (4 - cl0) * 512
            r = slice(qh * NH + qt * 128, qh * NH + qt * 128 + 128)
            rr = slice(qt * 128, qt * 128 + 128)
            M[r, qh * NH + cl0 * 512:(qh + 1) * NH] = R[rr, 0:w]
            M[r, (1 - qh) * NH + cl0 * 512:(2 - qh) * NH] = R[rr, w:2 * w]
    # mirror the missing block-lower-triangle tiles of every 2048x2048
    # quadrant from their transposed counterparts, then pin the exact
    # diagonal to exp(-dist(m,m)) = 1 (this also clears the NaNs that
    # sqrt of the diagonal's tiny negative psum noise produces)
    for bi in range(B):
        M = out[bi]
        for r0 in (0, NH):
            for c0 in (0, NH):
                for qt in range(NQT):
                    r = slice(r0 + qt * 128, r0 + qt * 128 + 128)
                    for cl in range(qt // 4):
                        cs = slice(c0 + cl * 512, c0 + cl * 512 + 512)
                        M[r, cs] = M[cs, r].T
        np.fill_diagonal(M, 1.0)
    return out, res


def kernel(x, W, b):
    out, _ = _run(x, W, b)
    return out


# revision 39
# speedup vs baseline: 1.0103x; 1.0103x over previous
"""Self-contained TRN2 Bass kernel for nn_EuclideanSimilarity.

Full-input contract: kernel(x, W, b) with
  x [4, 4096, 128] f32, W [128, 128] f32, b [128] f32
returns out [4, 4096, 4096] f32 = exp(-pairwise_euclidean_dist(x @ W.T + b)).

Sharding + symmetry: 8 cores, core c -> (batch c//2, half c%2).  Each batch's
similarity matrix is SYMMETRIC, so a core computes only the chunk-level upper
triangle of its two 2048x2048 quadrants: self (own-half queries x own-half
keys) and cross (own-half queries x other-half keys).  Core 2b covers D1 and
B-upper; core 2b+1 covers D2 and (B^T)-upper = B-lower.  The host mirrors the
missing block-lower tiles from their (always directly computed) transposed
counterparts.  Each core's key tensor is ordered [own half | other half], so
all 8 cores run an IDENTICAL program (SPMD); only the host-side column
mapping differs.  Per-core work: 80 of 128 [128,512] output chunks.

Numerics: x^T and W^T are prepared on the host (input marshalling) so the
device does no input transposes.  h^T = W @ x^T + b is computed per 512-token
chunk in fp32 and rounded ONCE to fp32r; queries are the self-half keys, so 8
projection matmuls cover everything.  The gram matmul of fp32r operands is
exact (24-bit products, fp32 accumulate) and the norms are computed from the
SAME rounded values, so d2 = sq[m] + sq[n] - 2 h_m.h_n is the exact pairwise
distance of the rounded vectors - no catastrophic cancellation; h_r vs h is a
~2^-12 relative perturbation, invisible through exp(-sqrt).  The exact
diagonal (true d2 = 0, computed 0 +/- psum accumulation noise, possibly NaN
after sqrt of a tiny negative) is pinned to exp(0) = 1 on the host during
unsharding, which also clears those NaNs.

Per [128,512] chunk the PE does TWO fp32r matmuls: gram (h queries x keys,
K=128) and a K=1 "aug" adding -sq_k[n]/2 (ones lhsT x norm-row rhs).  The
drain then computes d2 = -2*(psum) + sq_q[m]: for ACT-assigned qtiles as one
fused ACT op sqrt(-2*psum + sq_q) straight from PSUM (sqrt table), otherwise
as a DVE tensor_scalar into fp16 followed by sqrt as pow(x, 0.5) on the
GPSIMD (a third elementwise lane).  exp(-x) runs on ACT into bf16 (one exp
table load), leaves as bf16 (half DMA bytes, one packed DMA per qtile), and
is upcast on the host during unsharding.  Qtiles are processed 15->0 (small
ones first - they need only the first-loaded key chunks, so the main loop
starts ~5us in, overlapping the rest of the projection prologue).
"""

from contextlib import ExitStack

import numpy as np

import concourse.mybir as mybir
import concourse.tile as tile
from concourse.tile import add_dep_helper
from concourse import bacc
from concourse.bass import ts
from concourse.masks import make_identity

F32 = mybir.dt.float32
F32R = mybir.dt.float32r
F16 = mybir.dt.float16
BF16 = mybir.dt.bfloat16
AF = mybir.ActivationFunctionType
ALU = mybir.AluOpType

B = 4
N = 4096
NH = 2048   # tokens per half (queries per core)
D = 128
TEMPERATURE = 1.0
NQT = NH // 128  # 16 query tiles per core
N_CORES = 8

# qtiles whose drain+sqrt runs fused on ACT (sqrt table); the rest drain via
# DVE and take sqrt on the GPSIMD pow lane.  The small qtiles run first (they
# need only the first-loaded key chunks, overlapping the prologue), and one
# mid-size ACT qtile sits in the final group to balance the engines.
ACT_SQRT_QT = (15, 14, 13, 12, 7)


def kernel_body(ctx: ExitStack, tc: tile.TileContext, out, xt, wt, b):
    nc = tc.nc

    consts = ctx.enter_context(tc.tile_pool(name="consts", bufs=1))
    # preload the sqrt table set while the prologue runs
    scrap = consts.tile([1, 8], F32)
    nc.gpsimd.memset(scrap[:], 1.0)
    nc.scalar.activation(scrap[:], scrap[:], AF.Sqrt)

    ident = consts.tile([128, 128], F32)
    make_identity(nc, ident[:])

    wt_sb = consts.tile([128, 128], F32)
    nc.sync.dma_start(wt_sb[:], wt[:, :])
    b_sb = consts.tile([128, 1], F32)
    nc.sync.dma_start(b_sb[:], b[:, :])
    ones_col = consts.tile([128, 1], F32)    # lhsT for the f32 sq matmul
    nc.gpsimd.memset(ones_col[:], 1.0)
    ones_row = consts.tile([1, 128], F32)
    nc.gpsimd.memset(ones_row[:], 1.0)
    ones1r = consts.tile([1, 128], F32R)     # K=1 lhsT for the aug matmul
    nc.vector.tensor_copy(ones1r[:], ones_row[:])
    half16 = consts.tile([128, N], F16)      # pow exponent for gpsimd sqrt
    nc.vector.memset(half16[:], 0.5)

    # persistent operands
    h_pool = ctx.enter_context(tc.tile_pool(name="h", bufs=1))
    hk = h_pool.tile([128, N], F32R)         # h for all 4096 keys
    augk = h_pool.tile([1, N], F32R)         # -sq_k/2 row
    sqq_cols = h_pool.tile([128, NQT], F32)  # sq_q column per qtile
    sq_row = h_pool.tile([1, NH], F32)       # self-half norms (fp32)

    # main-loop pools created first so the (scoped) projection pools sit on
    # top of the pool stack and can be released before the tail group
    st_pool = ctx.enter_context(tc.tile_pool(name="st", bufs=6))
    st2_pool = ctx.enter_context(tc.tile_pool(name="st2", bufs=6))
    ps_pool = ctx.enter_context(tc.tile_pool(name="d2", bufs=2, space="PSUM"))

    # projection pools (coexist with the early main loop; 4 PSUM banks,
    # closed after the last chunk so the tail group gets a second psum pool)
    setup_ctx = ExitStack()
    ssb = setup_ctx.enter_context(tc.tile_pool(name="setup_sb", bufs=3))
    sps = setup_ctx.enter_context(tc.tile_pool(name="setup_ps", bufs=1, space="PSUM"))

    # PE p-state warmup: ~60 tiny matmuls keep the tensor engine busy from
    # t~1us so its clock is fully ramped (3us of continuous execution) by the
    # time the first real fp32 projection matmul dispatches — those would
    # otherwise run 2-3x slower and serialize the whole prologue.
    warm = sps.tile([128, 16], F32, tag="sqqtp", bufs=1, name="warm")
    for i in range(60):
        nc.tensor.matmul(warm[:], ident[:], ident[:, 0:16],
                         start=True, stop=True)

    # prefetch all key chunks up front (the per-chunk DMA latency otherwise
    # serializes into the projection chain)
    xins = {}
    for c in (3, 7, 2, 6, 1, 5, 0, 4):
        xin = ssb.tile([128, 512], F32, tag="xin", bufs=8, name=f"xin{c}")
        nc.sync.dma_start(xin[:], xt[:, ts(c, 512)])
        xins[c] = xin

    s2fs = {}

    def project_mm(c):
        hps = sps.tile([128, 512], F32, tag="hps", bufs=2, name=f"hps{c}")
        nc.tensor.matmul(hps[:], wt_sb[:], xins[c][:], start=True, stop=True)
        nc.vector.tensor_scalar_add(hk[:, ts(c, 512)], hps[:], b_sb[:, 0:1])
        s2f = ssb.tile([128, 512], F32, tag="s2f", bufs=8, name=f"s2f{c}")
        nc.vector.tensor_mul(s2f[:], hk[:, ts(c, 512)], hk[:, ts(c, 512)])
        s2fs[c] = s2f

    def project_norms(c):
        sqps = sps.tile([128, 512], F32, tag="sqps", bufs=1, name=f"sqps{c}")
        nc.tensor.matmul(
            sqps[0:1, :], ones_col[:], s2fs.pop(c)[:], start=True, stop=True
        )
        nc.scalar.activation(
            augk[0:1, ts(c, 512)], sqps[0:1, :], AF.Identity, scale=-0.5
        )
        if c < NH // 512:  # self chunk: also query norms (exact fp32)
            nc.vector.tensor_copy(sq_row[0:1, ts(c, 512)], sqps[0:1, :])
            for j in range(4):
                qt = 4 * c + j
                nc.tensor.transpose(
                    warm[:, qt % 4:qt % 4 + 1], sq_row[0:1, ts(qt, 128)],
                    ident[0:1, 0:1],
                )
            nc.vector.tensor_copy(
                sqq_cols[:, 4 * c:4 * c + 4], warm[:, 0:4]
            )

    def project_chunk(c):
        project_mm(c)
        project_norms(c)

    # ---------------- main loop ----------------
    last_act = [None]

    def chained_act(*args, **kwargs):
        bi = nc.scalar.activation(*args, **kwargs)
        if last_act[0] is not None:
            add_dep_helper(bi.ins, last_act[0].ins, sync=False,
                           reason="act-table-order")
        last_act[0] = bi
        return bi

    pending = []

    def emit_tail(qt, st, nch, split):
        st2 = st2_pool.tile([128, nch * 512], BF16, tag="st2", name=f"st2_{qt}")
        # NaNs from sqrt of tiny diagonal negatives pass through; the host
        # pins the exact diagonal to exp(0)=1 afterwards.
        if split:  # final qtile: halve exp+DMA so the last DMA overlaps
            h = nch * 256
            chained_act(st2[:, 0:h], st[:, 0:h], AF.Exp, scale=-TEMPERATURE)
            nc.sync.dma_start(out[ts(qt, 128), 0:h], st2[:, 0:h])
            chained_act(st2[:, h:2 * h], st[:, h:2 * h], AF.Exp,
                        scale=-TEMPERATURE)
            nc.sync.dma_start(out[ts(qt, 128), h:2 * h], st2[:, h:2 * h])
        else:
            chained_act(st2[:], st[:], AF.Exp, scale=-TEMPERATURE)
            nc.sync.dma_start(out[ts(qt, 128), 0:nch * 512], st2[:])

    def emit_qtile(qt, last=False, pool=None):
        cl0 = qt // 4
        nch = 2 * (4 - cl0)
        cols = [cl * 512 for cl in range(cl0, 4)] + \
               [NH + cl * 512 for cl in range(cl0, 4)]
        st = st_pool.tile([128, nch * 512], F16, tag="st", name=f"st{qt}")
        for s0 in range(0, nch, 2):
            seg = cols[s0:s0 + 2]
            ps = (pool or ps_pool).tile(
                [128, 1024], F32, tag="d2", name=f"d2_{qt}_{s0}")
            for j, col in enumerate(seg):
                psl = ps[:, ts(j, 512)]
                nc.tensor.matmul(
                    psl, hk[:, ts(qt, 128)], hk[:, col:col + 512],
                    start=True, stop=False,
                )
                nc.tensor.matmul(
                    psl, ones1r[:], augk[:, col:col + 512],
                    start=False, stop=True,
                )
            sw = len(seg) * 512
            sl = st[:, s0 * 512:s0 * 512 + sw]
            if qt in ACT_SQRT_QT:
                # fused drain+sqrt: sqrt(-2*psum + sq_q)
                chained_act(sl, ps[:, 0:sw], AF.Sqrt,
                            bias=sqq_cols[:, qt:qt + 1], scale=-2.0)
            else:
                # d2 = -2*psum + sq_q
                nc.vector.tensor_scalar(
                    sl, ps[:, 0:sw], -2.0, sqq_cols[:, qt:qt + 1],
                    ALU.mult, ALU.add,
                )
                if last or nch == 8:  # seg-granular sqrt: shorter chain
                    nc.gpsimd.tensor_tensor(
                        sl, sl, half16[:, 0:sw], ALU.pow
                    )
        if qt not in ACT_SQRT_QT and not last and nch != 8:
            nc.gpsimd.tensor_tensor(
                st[:], st[:], half16[:, 0:nch * 512], ALU.pow
            )
        pending.append((qt, st, nch, last))
        if qt == 12:  # ACT sqrt-phase over; exps flow from here (1 exp load)
            for item in pending:
                emit_tail(*item)
            pending.clear()
        elif qt < 12:
            emit_tail(*pending.pop())

    # project chunks 3,7 first (the small qtiles need only those), then the
    # ACT-drained small qtiles stream while the remaining chunks project on
    # the warmed PE; after that the whole schedule is projection-free
    project_mm(3)
    project_mm(7)
    project_norms(3)
    project_norms(7)
    for qt in (15, 14, 13, 12):
        emit_qtile(qt)
    project_mm(2)
    project_mm(6)
    project_norms(2)
    project_norms(6)
    for qt in (11, 10, 9, 8):
        emit_qtile(qt)
    project_mm(1)
    project_mm(5)
    project_norms(1)
    project_norms(5)
    project_mm(0)
    project_mm(4)
    project_norms(0)
    project_norms(4)
    setup_ctx.close()
    ps_pool2 = ctx.enter_context(tc.tile_pool(name="d2b", bufs=2, space="PSUM"))
    # qt7 (the ACT-drained one) first in the tail group so its PSUM tiles
    # recycle early instead of queueing behind the exp chain; alternate psum
    # pools for double pipeline depth
    for i, qt in enumerate((7, 3, 2, 1, 0, 6, 5)):
        emit_qtile(qt, pool=ps_pool2 if i % 2 else None)
    emit_qtile(4, last=True, pool=ps_pool2)


def build_nc():
    nc = bacc.Bacc("TRN2", target_bir_lowering=False, debug=False)
    xt = nc.dram_tensor("xt", [D, N], F32, kind="ExternalInput").ap()
    wt = nc.dram_tensor("wt", [D, D], F32, kind="ExternalInput").ap()
    b = nc.dram_tensor("b", [D, 1], F32, kind="ExternalInput").ap()
    out = nc.dram_tensor("out", [NH, N], BF16, kind="ExternalOutput").ap()
    with tile.TileContext(nc) as tc:
        with ExitStack() as ctx:
            kernel_body(ctx, tc, out, xt, wt, b)
    nc.compile()
    return nc


_NC_CACHE = None


def _get_nc():
    global _NC_CACHE
    if _NC_CACHE is None:
        _NC_CACHE = build_nc()
    return _NC_CACHE


def _run(x, W, b, trace=False, **spmd_kwargs):
    from concourse.bass_utils import run_bass_kernel_spmd

    x = np.asarray(x, dtype=np.float32)
    W = np.asarray(W, dtype=np.float32)
    b = np.asarray(b, dtype=np.float32).reshape(D, 1)
    wt = np.ascontiguousarray(W.T)
    nc = _get_nc()
    in_maps = []
    for c in range(N_CORES):
        bi, qh = c // 2, c % 2
        xtb = x[bi].T  # [128, 4096]
        own = xtb[:, qh * NH:(qh + 1) * NH]
        oth = xtb[:, (1 - qh) * NH:(2 - qh) * NH]
        in_maps.append({
            "xt": np.ascontiguousarray(np.concatenate([own, oth], axis=1)),
            "wt": wt,
            "b": b,
        })
    res = run_bass_kernel_spmd(
        nc, in_maps, core_ids=list(range(N_CORES)), trace=trace, **spmd_kwargs
    )
    out = np.empty((B, N, N), dtype=np.float32)
    for c in range(N_CORES):
        bi, qh = c // 2, c % 2
        R = res.results[c]["out"].astype(np.float32)  # [2048, 4096] packed
        rows = slice(qh * NH, (qh + 1) * NH)
        M = out[bi]
        for qt in range(NQT):
            cl0 = qt // 4
            w = (4 - cl0) * 512
            r = slice(qh * NH + qt * 128, qh * NH + qt * 128 + 128)
            rr = slice(qt * 128, qt * 128 + 128)
            M[r, qh * NH + cl0 * 512:(qh + 1) * NH] = R[rr, 0:w]
            M[r, (1 - qh) * NH + cl0 * 512:(2 - qh) * NH] = R[rr, w:2 * w]
    # mirror the missing block-lower-triangle tiles of every 2048x2048
    # quadrant from their transposed counterparts, then pin the exact
    # diagonal to exp(-dist(m,m)) = 1 (this also clears the NaNs that
    # sqrt of the diagonal's tiny negative psum noise produces)
    for bi in range(B):
        M = out[bi]
        for r0 in (0, NH):
            for c0 in (0, NH):
                for qt in range(NQT):
                    r = slice(r0 + qt * 128, r0 + qt * 128 + 128)
                    for cl in range(qt // 4):
                        cs = slice(c0 + cl * 512, c0 + cl * 512 + 512)
                        M[r, cs] = M[cs, r].T
        np.fill_diagonal(M, 1.0)
    return out, res


def kernel(x, W, b):
    out, _ = _run(x, W, b)
    return out


# revision 40
# speedup vs baseline: 1.0977x; 1.0865x over previous
"""Self-contained TRN2 Bass kernel for nn_EuclideanSimilarity.

Full-input contract: kernel(x, W, b) with
  x [4, 4096, 128] f32, W [128, 128] f32, b [128] f32
returns out [4, 4096, 4096] f32 = exp(-pairwise_euclidean_dist(x @ W.T + b)).

Sharding + symmetry: 8 cores, core c -> (batch c//2, half c%2).  Each batch's
similarity matrix is SYMMETRIC, so a core computes only the chunk-level upper
triangle of its two 2048x2048 quadrants: self (own-half queries x own-half
keys) and cross (own-half queries x other-half keys).  Core 2b covers D1 and
B-upper; core 2b+1 covers D2 and (B^T)-upper = B-lower.  The host mirrors the
missing block-lower tiles from their (always directly computed) transposed
counterparts.  Each core's key tensor is ordered [own half | other half], so
all 8 cores run an IDENTICAL program (SPMD); only the host-side column
mapping differs.  Per-core work: 80 of 128 [128,512] output chunks.

Numerics: x^T and W^T are prepared on the host (input marshalling) so the
device does no input transposes.  h^T = W @ x^T + b is computed per 512-token
chunk in fp32 and rounded ONCE to fp32r; queries are the self-half keys, so 8
projection matmuls cover everything.  The gram matmul of fp32r operands is
exact (24-bit products, fp32 accumulate) and the norms are computed from the
SAME rounded values, so d2 = sq[m] + sq[n] - 2 h_m.h_n is the exact pairwise
distance of the rounded vectors - no catastrophic cancellation; h_r vs h is a
~2^-12 relative perturbation, invisible through exp(-sqrt).  The exact
diagonal (true d2 = 0, computed 0 +/- psum accumulation noise, possibly NaN
after sqrt of a tiny negative) is pinned to exp(0) = 1 on the host during
unsharding, which also clears those NaNs.

Per [128,512] chunk the PE does TWO fp32r matmuls: gram (h queries x keys,
K=128) and a K=1 "aug" adding -sq_k[n]/2 (ones lhsT x norm-row rhs).  The
drain then computes d2 = -2*(psum) + sq_q[m]: for ACT-assigned qtiles as one
fused ACT op sqrt(-2*psum + sq_q) straight from PSUM (sqrt table), otherwise
as a DVE tensor_scalar into fp16 followed by sqrt as pow(x, 0.5) on the
GPSIMD (a third elementwise lane).  exp(-x) runs on ACT into bf16 (one exp
table load), leaves as bf16 (half DMA bytes, one packed DMA per qtile), and
is upcast on the host during unsharding.  Qtiles are processed 15->0 (small
ones first - they need only the first-loaded key chunks, so the main loop
starts ~5us in, overlapping the rest of the projection prologue).
"""

from contextlib import ExitStack

import numpy as np

import concourse.mybir as mybir
import concourse.tile as tile
from concourse.tile import add_dep_helper
from concourse import bacc
from concourse.bass import ts
from concourse.masks import make_identity

F32 = mybir.dt.float32
F32R = mybir.dt.float32r
F16 = mybir.dt.float16
BF16 = mybir.dt.bfloat16
AF = mybir.ActivationFunctionType
ALU = mybir.AluOpType

B = 4
N = 4096
NH = 2048   # tokens per half (queries per core)
D = 128
TEMPERATURE = 1.0
NQT = NH // 128  # 16 query tiles per core
N_CORES = 8

# qtiles whose drain+sqrt runs fused on ACT (sqrt table); the rest drain via
# DVE and take sqrt on the GPSIMD pow lane.  The small qtiles run first (they
# need only the first-loaded key chunks, overlapping the prologue), and one
# mid-size ACT qtile sits in the final group to balance the engines.
ACT_SQRT_QT = (15, 7)


def kernel_body(ctx: ExitStack, tc: tile.TileContext, out, xt, wt, b):
    nc = tc.nc

    consts = ctx.enter_context(tc.tile_pool(name="consts", bufs=1))
    # preload the sqrt table set while the prologue runs
    scrap = consts.tile([1, 8], F32)
    nc.gpsimd.memset(scrap[:], 1.0)
    nc.scalar.activation(scrap[:], scrap[:], AF.Sqrt)

    ident = consts.tile([128, 128], F32)
    make_identity(nc, ident[:])

    wt_sb = consts.tile([128, 128], F32)
    nc.sync.dma_start(wt_sb[:], wt[:, :])
    b_sb = consts.tile([128, 1], F32)
    nc.sync.dma_start(b_sb[:], b[:, :])
    ones_col = consts.tile([128, 1], F32)    # lhsT for the f32 sq matmul
    nc.gpsimd.memset(ones_col[:], 1.0)
    ones_row = consts.tile([1, 128], F32)
    nc.gpsimd.memset(ones_row[:], 1.0)
    ones1r = consts.tile([1, 128], F32R)     # K=1 lhsT for the aug matmul
    nc.vector.tensor_copy(ones1r[:], ones_row[:])
    half16 = consts.tile([128, N], F16)      # pow exponent for gpsimd sqrt
    nc.vector.memset(half16[:], 0.5)

    # persistent operands
    h_pool = ctx.enter_context(tc.tile_pool(name="h", bufs=1))
    hk = h_pool.tile([128, N], F32R)         # h for all 4096 keys
    augk = h_pool.tile([1, N], F32R)         # -sq_k/2 row
    sqq_cols = h_pool.tile([128, NQT], F32)  # sq_q column per qtile
    sq_row = h_pool.tile([1, NH], F32)       # self-half norms (fp32)

    # main-loop pools created first so the (scoped) projection pools sit on
    # top of the pool stack and can be released before the tail group
    st_pool = ctx.enter_context(tc.tile_pool(name="st", bufs=6))
    st2_pool = ctx.enter_context(tc.tile_pool(name="st2", bufs=6))
    ps_pool = ctx.enter_context(tc.tile_pool(name="d2", bufs=2, space="PSUM"))

    # projection pools (coexist with the early main loop; 4 PSUM banks,
    # closed after the last chunk so the tail group gets a second psum pool)
    setup_ctx = ExitStack()
    ssb = setup_ctx.enter_context(tc.tile_pool(name="setup_sb", bufs=3))
    sps = setup_ctx.enter_context(tc.tile_pool(name="setup_ps", bufs=1, space="PSUM"))

    # PE p-state warmup: ~60 tiny matmuls keep the tensor engine busy from
    # t~1us so its clock is fully ramped (3us of continuous execution) by the
    # time the first real fp32 projection matmul dispatches — those would
    # otherwise run 2-3x slower and serialize the whole prologue.
    warm = sps.tile([128, 16], F32, tag="sqqtp", bufs=1, name="warm")
    for i in range(60):
        nc.tensor.matmul(warm[:], ident[:], ident[:, 0:16],
                         start=True, stop=True)

    # prefetch all key chunks up front (the per-chunk DMA latency otherwise
    # serializes into the projection chain)
    xins = {}
    for c in (3, 7, 2, 6, 1, 5, 0, 4):
        xin = ssb.tile([128, 512], F32, tag="xin", bufs=8, name=f"xin{c}")
        nc.sync.dma_start(xin[:], xt[:, ts(c, 512)])
        xins[c] = xin

    s2fs = {}

    def project_mm(c):
        hps = sps.tile([128, 512], F32, tag="hps", bufs=2, name=f"hps{c}")
        nc.tensor.matmul(hps[:], wt_sb[:], xins[c][:], start=True, stop=True)
        nc.vector.tensor_scalar_add(hk[:, ts(c, 512)], hps[:], b_sb[:, 0:1])
        s2f = ssb.tile([128, 512], F32, tag="s2f", bufs=8, name=f"s2f{c}")
        nc.vector.tensor_mul(s2f[:], hk[:, ts(c, 512)], hk[:, ts(c, 512)])
        s2fs[c] = s2f

    def project_norms(c):
        sqps = sps.tile([128, 512], F32, tag="sqps", bufs=1, name=f"sqps{c}")
        nc.tensor.matmul(
            sqps[0:1, :], ones_col[:], s2fs.pop(c)[:], start=True, stop=True
        )
        nc.scalar.activation(
            augk[0:1, ts(c, 512)], sqps[0:1, :], AF.Identity, scale=-0.5
        )
        if c < NH // 512:  # self chunk: also query norms (exact fp32)
            nc.vector.tensor_copy(sq_row[0:1, ts(c, 512)], sqps[0:1, :])
            for j in range(4):
                qt = 4 * c + j
                nc.tensor.transpose(
                    warm[:, qt % 4:qt % 4 + 1], sq_row[0:1, ts(qt, 128)],
                    ident[0:1, 0:1],
                )
            nc.vector.tensor_copy(
                sqq_cols[:, 4 * c:4 * c + 4], warm[:, 0:4]
            )

    def project_chunk(c):
        project_mm(c)
        project_norms(c)

    # ---------------- main loop ----------------
    last_act = [None]

    def chained_act(*args, **kwargs):
        bi = nc.scalar.activation(*args, **kwargs)
        if last_act[0] is not None:
            add_dep_helper(bi.ins, last_act[0].ins, sync=False,
                           reason="act-table-order")
        last_act[0] = bi
        return bi

    pending = []

    def emit_tail(qt, st, nch, split):
        st2 = st2_pool.tile([128, nch * 512], BF16, tag="st2", name=f"st2_{qt}")
        # NaNs from sqrt of tiny diagonal negatives pass through; the host
        # pins the exact diagonal to exp(0)=1 afterwards.
        if split:  # final qtile: halve exp+DMA so the last DMA overlaps
            h = nch * 256
            chained_act(st2[:, 0:h], st[:, 0:h], AF.Exp, scale=-TEMPERATURE)
            nc.sync.dma_start(out[ts(qt, 128), 0:h], st2[:, 0:h])
            chained_act(st2[:, h:2 * h], st[:, h:2 * h], AF.Exp,
                        scale=-TEMPERATURE)
            nc.sync.dma_start(out[ts(qt, 128), h:2 * h], st2[:, h:2 * h])
        else:
            chained_act(st2[:], st[:], AF.Exp, scale=-TEMPERATURE)
            nc.sync.dma_start(out[ts(qt, 128), 0:nch * 512], st2[:])

    def emit_qtile(qt, last=False, pool=None):
        cl0 = qt // 4
        nch = 2 * (4 - cl0)
        cols = [cl * 512 for cl in range(cl0, 4)] + \
               [NH + cl * 512 for cl in range(cl0, 4)]
        st = st_pool.tile([128, nch * 512], F16, tag="st", name=f"st{qt}")
        for s0 in range(0, nch, 2):
            seg = cols[s0:s0 + 2]
            ps = (pool or ps_pool).tile(
                [128, 1024], F32, tag="d2", name=f"d2_{qt}_{s0}")
            for j, col in enumerate(seg):
                psl = ps[:, ts(j, 512)]
                nc.tensor.matmul(
                    psl, hk[:, ts(qt, 128)], hk[:, col:col + 512],
                    start=True, stop=False,
                )
                nc.tensor.matmul(
                    psl, ones1r[:], augk[:, col:col + 512],
                    start=False, stop=True,
                )
            sw = len(seg) * 512
            sl = st[:, s0 * 512:s0 * 512 + sw]
            if qt in ACT_SQRT_QT:
                # fused drain+sqrt: sqrt(-2*psum + sq_q)
                chained_act(sl, ps[:, 0:sw], AF.Sqrt,
                            bias=sqq_cols[:, qt:qt + 1], scale=-2.0)
            else:
                # d2 = -2*psum + sq_q
                nc.vector.tensor_scalar(
                    sl, ps[:, 0:sw], -2.0, sqq_cols[:, qt:qt + 1],
                    ALU.mult, ALU.add,
                )
                if last or nch == 8:  # seg-granular sqrt: shorter chain
                    nc.gpsimd.tensor_tensor(
                        sl, sl, half16[:, 0:sw], ALU.pow
                    )
        if qt not in ACT_SQRT_QT and not last and nch != 8:
            nc.gpsimd.tensor_tensor(
                st[:], st[:], half16[:, 0:nch * 512], ALU.pow
            )
        pending.append((qt, st, nch, last))
        if qt == 15:
            for item in pending:
                emit_tail(*item)
            pending.clear()
        else:
            emit_tail(*pending.pop())

    # project chunks 3,7 first (the small qtiles need only those), then the
    # ACT-drained small qtiles stream while the remaining chunks project on
    # the warmed PE; after that the whole schedule is projection-free
    project_mm(3)
    project_mm(7)
    project_norms(3)
    project_norms(7)
    for qt in (15, 14, 13, 12):
        emit_qtile(qt)
    project_mm(2)
    project_mm(6)
    project_norms(2)
    project_norms(6)
    for qt in (11, 10, 9, 8):
        emit_qtile(qt)
    project_mm(1)
    project_mm(5)
    project_norms(1)
    project_norms(5)
    project_mm(0)
    project_mm(4)
    project_norms(0)
    project_norms(4)
    setup_ctx.close()
    ps_pool2 = ctx.enter_context(tc.tile_pool(name="d2b", bufs=2, space="PSUM"))
    # qt7 (the ACT-drained one) first in the tail group so its PSUM tiles
    # recycle early instead of queueing behind the exp chain; alternate psum
    # pools for double pipeline depth
    for i, qt in enumerate((7, 3, 2, 1, 0, 6, 5)):
        emit_qtile(qt, pool=ps_pool2 if i % 2 else None)
    emit_qtile(4, last=True, pool=ps_pool2)


def build_nc():
    nc = bacc.Bacc("TRN2", target_bir_lowering=False, debug=False)
    xt = nc.dram_tensor("xt", [D, N], F32, kind="ExternalInput").ap()
    wt = nc.dram_tensor("wt", [D, D], F32, kind="ExternalInput").ap()
    b = nc.dram_tensor("b", [D, 1], F32, kind="ExternalInput").ap()
    out = nc.dram_tensor("out", [NH, N], BF16, kind="ExternalOutput").ap()
    with tile.TileContext(nc) as tc:
        with ExitStack() as ctx:
            kernel_body(ctx, tc, out, xt, wt, b)
    nc.compile()
    return nc


_NC_CACHE = None


def _get_nc():
    global _NC_CACHE
    if _NC_CACHE is None:
        _NC_CACHE = build_nc()
    return _NC_CACHE


def _run(x, W, b, trace=False, **spmd_kwargs):
    from concourse.bass_utils import run_bass_kernel_spmd

    x = np.asarray(x, dtype=np.float32)
    W = np.asarray(W, dtype=np.float32)
    b = np.asarray(b, dtype=np.float32).reshape(D, 1)
    wt = np.ascontiguousarray(W.T)
    nc = _get_nc()
    in_maps = []
    for c in range(N_CORES):
        bi, qh = c // 2, c % 2
        xtb = x[bi].T  # [128, 4096]
        own = xtb[:, qh * NH:(qh + 1) * NH]
        oth = xtb[:, (1 - qh) * NH:(2 - qh) * NH]
        in_maps.append({
            "xt": np.ascontiguousarray(np.concatenate([own, oth], axis=1)),
            "wt": wt,
            "b": b,
        })
    res = run_bass_kernel_spmd(
        nc, in_maps, core_ids=list(range(N_CORES)), trace=trace, **spmd_kwargs
    )
    out = np.empty((B, N, N), dtype=np.float32)
    for c in range(N_CORES):
        bi, qh = c // 2, c % 2
        R = res.results[c]["out"].astype(np.float32)  # [2048, 4096] packed
        rows = slice(qh * NH, (qh + 1) * NH)
        M = out[bi]
        for qt in range(NQT):
            cl0 = qt // 4
            w = (4 - cl0) * 512
            r = slice(qh * NH + qt * 128, qh * NH + qt * 128 + 128)
            rr = slice(qt * 128, qt * 128 + 128)
            M[r, qh * NH + cl0 * 512:(qh + 1) * NH] = R[rr, 0:w]
            M[r, (1 - qh) * NH + cl0 * 512:(2 - qh) * NH] = R[rr, w:2 * w]
    # mirror the missing block-lower-triangle tiles of every 2048x2048
    # quadrant from their transposed counterparts, then pin the exact
    # diagonal to exp(-dist(m,m)) = 1 (this also clears the NaNs that
    # sqrt of the diagonal's tiny negative psum noise produces)
    for bi in range(B):
        M = out[bi]
        for r0 in (0, NH):
            for c0 in (0, NH):
                for qt in range(NQT):
                    r = slice(r0 + qt * 128, r0 + qt * 128 + 128)
                    for cl in range(qt // 4):
                        cs = slice(c0 + cl * 512, c0 + cl * 512 + 512)
                        M[r, cs] = M[cs, r].T
        np.fill_diagonal(M, 1.0)
    return out, res


def kernel(x, W, b):
    out, _ = _run(x, W, b)
    return out


# revision 41
# speedup vs baseline: 1.1287x; 1.0282x over previous
"""Self-contained TRN2 Bass kernel for nn_EuclideanSimilarity.

Full-input contract: kernel(x, W, b) with
  x [4, 4096, 128] f32, W [128, 128] f32, b [128] f32
returns out [4, 4096, 4096] f32 = exp(-pairwise_euclidean_dist(x @ W.T + b)).

Sharding + symmetry: 8 cores, core c -> (batch c//2, half c%2).  Each batch's
similarity matrix is SYMMETRIC, so a core computes only the chunk-level upper
triangle of its two 2048x2048 quadrants: self (own-half queries x own-half
keys) and cross (own-half queries x other-half keys).  Core 2b covers D1 and
B-upper; core 2b+1 covers D2 and (B^T)-upper = B-lower.  The host mirrors the
missing block-lower tiles from their (always directly computed) transposed
counterparts.  Each core's key tensor is ordered [own half | other half], so
all 8 cores run an IDENTICAL program (SPMD); only the host-side column
mapping differs.  Per-core work: 80 of 128 [128,512] output chunks.

Numerics: x^T and W^T are prepared on the host (input marshalling) so the
device does no input transposes.  h^T = W @ x^T + b is computed per 512-token
chunk in fp32 and rounded ONCE to fp32r; queries are the self-half keys, so 8
projection matmuls cover everything.  The gram matmul of fp32r operands is
exact (24-bit products, fp32 accumulate) and the norms are computed from the
SAME rounded values, so d2 = sq[m] + sq[n] - 2 h_m.h_n is the exact pairwise
distance of the rounded vectors - no catastrophic cancellation; h_r vs h is a
~2^-12 relative perturbation, invisible through exp(-sqrt).  The exact
diagonal (true d2 = 0, computed 0 +/- psum accumulation noise, possibly NaN
after sqrt of a tiny negative) is pinned to exp(0) = 1 on the host during
unsharding, which also clears those NaNs.

Per [128,512] chunk the PE does TWO fp32r matmuls: gram (h queries x keys,
K=128) and a K=1 "aug" adding -sq_k[n]/2 (ones lhsT x norm-row rhs).  The
drain then computes d2 = -2*(psum) + sq_q[m]: for ACT-assigned qtiles as one
fused ACT op sqrt(-2*psum + sq_q) straight from PSUM (sqrt table), otherwise
as a DVE tensor_scalar into fp16 followed by sqrt as pow(x, 0.5) on the
GPSIMD (a third elementwise lane).  exp(-x) runs on ACT into bf16 (one exp
table load), leaves as bf16 (half DMA bytes, one packed DMA per qtile), and
is upcast on the host during unsharding.  Qtiles are processed 15->0 (small
ones first - they need only the first-loaded key chunks, so the main loop
starts ~5us in, overlapping the rest of the projection prologue).
"""

from contextlib import ExitStack

import numpy as np

import concourse.mybir as mybir
import concourse.tile as tile
from concourse.tile import add_dep_helper
from concourse import bacc
from concourse.bass import ts
from concourse.masks import make_identity

F32 = mybir.dt.float32
F32R = mybir.dt.float32r
F16 = mybir.dt.float16
BF16 = mybir.dt.bfloat16
AF = mybir.ActivationFunctionType
ALU = mybir.AluOpType

B = 4
N = 4096
NH = 2048   # tokens per half (queries per core)
D = 128
TEMPERATURE = 1.0
NQT = NH // 128  # 16 query tiles per core
N_CORES = 8

# qtiles whose drain+sqrt runs fused on ACT (sqrt table); the rest drain via
# DVE and take sqrt on the GPSIMD pow lane.  The small qtiles run first (they
# need only the first-loaded key chunks, overlapping the prologue), and one
# mid-size ACT qtile sits in the final group to balance the engines.
ACT_SQRT_QT = (15, 8, 7)


def kernel_body(ctx: ExitStack, tc: tile.TileContext, out, xt, wt, b):
    nc = tc.nc

    consts = ctx.enter_context(tc.tile_pool(name="consts", bufs=1))
    # preload the sqrt table set while the prologue runs
    scrap = consts.tile([1, 8], F32)
    nc.gpsimd.memset(scrap[:], 1.0)
    nc.scalar.activation(scrap[:], scrap[:], AF.Sqrt)

    ident = consts.tile([128, 128], F32)
    make_identity(nc, ident[:])

    wt_sb = consts.tile([128, 128], F32)
    nc.sync.dma_start(wt_sb[:], wt[:, :])
    b_sb = consts.tile([128, 1], F32)
    nc.sync.dma_start(b_sb[:], b[:, :])
    ones_col = consts.tile([128, 1], F32)    # lhsT for the f32 sq matmul
    nc.gpsimd.memset(ones_col[:], 1.0)
    ones_row = consts.tile([1, 128], F32)
    nc.gpsimd.memset(ones_row[:], 1.0)
    ones1r = consts.tile([1, 128], F32R)     # K=1 lhsT for the aug matmul
    nc.vector.tensor_copy(ones1r[:], ones_row[:])
    half16 = consts.tile([128, N], F16)      # pow exponent for gpsimd sqrt
    nc.vector.memset(half16[:], 0.5)

    # persistent operands
    h_pool = ctx.enter_context(tc.tile_pool(name="h", bufs=1))
    hk = h_pool.tile([128, N], F32R)         # h for all 4096 keys
    augk = h_pool.tile([1, N], F32R)         # -sq_k/2 row
    sqq_cols = h_pool.tile([128, NQT], F32)  # sq_q column per qtile
    sq_row = h_pool.tile([1, NH], F32)       # self-half norms (fp32)

    # main-loop pools created first so the (scoped) projection pools sit on
    # top of the pool stack and can be released before the tail group
    st_pool = ctx.enter_context(tc.tile_pool(name="st", bufs=7))
    st2_pool = ctx.enter_context(tc.tile_pool(name="st2", bufs=7))
    ps_pool = ctx.enter_context(tc.tile_pool(name="d2", bufs=2, space="PSUM"))

    # projection pools (coexist with the early main loop; 4 PSUM banks,
    # closed after the last chunk so the tail group gets a second psum pool)
    setup_ctx = ExitStack()
    ssb = setup_ctx.enter_context(tc.tile_pool(name="setup_sb", bufs=3))
    sps = setup_ctx.enter_context(tc.tile_pool(name="setup_ps", bufs=1, space="PSUM"))

    # PE p-state warmup: ~60 tiny matmuls keep the tensor engine busy from
    # t~1us so its clock is fully ramped (3us of continuous execution) by the
    # time the first real fp32 projection matmul dispatches — those would
    # otherwise run 2-3x slower and serialize the whole prologue.
    warm = sps.tile([128, 16], F32, tag="sqqtp", bufs=1, name="warm")
    for i in range(60):
        nc.tensor.matmul(warm[:], ident[:], ident[:, 0:16],
                         start=True, stop=True)

    # prefetch all key chunks up front (the per-chunk DMA latency otherwise
    # serializes into the projection chain)
    xins = {}
    for c in (3, 7, 2, 6, 1, 5, 0, 4):
        xin = ssb.tile([128, 512], F32, tag="xin", bufs=8, name=f"xin{c}")
        nc.sync.dma_start(xin[:], xt[:, ts(c, 512)])
        xins[c] = xin

    s2fs = {}

    def project_mm(c):
        hps = sps.tile([128, 512], F32, tag="hps", bufs=2, name=f"hps{c}")
        nc.tensor.matmul(hps[:], wt_sb[:], xins[c][:], start=True, stop=True)
        nc.vector.tensor_scalar_add(hk[:, ts(c, 512)], hps[:], b_sb[:, 0:1])
        s2f = ssb.tile([128, 512], F32, tag="s2f", bufs=8, name=f"s2f{c}")
        nc.vector.tensor_mul(s2f[:], hk[:, ts(c, 512)], hk[:, ts(c, 512)])
        s2fs[c] = s2f

    def project_norms(c):
        sqps = sps.tile([128, 512], F32, tag="sqps", bufs=1, name=f"sqps{c}")
        nc.tensor.matmul(
            sqps[0:1, :], ones_col[:], s2fs.pop(c)[:], start=True, stop=True
        )
        nc.scalar.activation(
            augk[0:1, ts(c, 512)], sqps[0:1, :], AF.Identity, scale=-0.5
        )
        if c < NH // 512:  # self chunk: also query norms (exact fp32)
            nc.vector.tensor_copy(sq_row[0:1, ts(c, 512)], sqps[0:1, :])
            for j in range(4):
                qt = 4 * c + j
                nc.tensor.transpose(
                    warm[:, qt % 4:qt % 4 + 1], sq_row[0:1, ts(qt, 128)],
                    ident[0:1, 0:1],
                )
            nc.vector.tensor_copy(
                sqq_cols[:, 4 * c:4 * c + 4], warm[:, 0:4]
            )

    def project_chunk(c):
        project_mm(c)
        project_norms(c)

    # ---------------- main loop ----------------
    last_act = [None]

    def chained_act(*args, **kwargs):
        bi = nc.scalar.activation(*args, **kwargs)
        if last_act[0] is not None:
            add_dep_helper(bi.ins, last_act[0].ins, sync=False,
                           reason="act-table-order")
        last_act[0] = bi
        return bi

    pending = []

    def emit_tail(qt, st, nch, split):
        st2 = st2_pool.tile([128, nch * 512], BF16, tag="st2", name=f"st2_{qt}")
        # NaNs from sqrt of tiny diagonal negatives pass through; the host
        # pins the exact diagonal to exp(0)=1 afterwards.
        if split:  # final qtile: halve exp+DMA so the last DMA overlaps
            h = nch * 256
            chained_act(st2[:, 0:h], st[:, 0:h], AF.Exp, scale=-TEMPERATURE)
            nc.sync.dma_start(out[ts(qt, 128), 0:h], st2[:, 0:h])
            chained_act(st2[:, h:2 * h], st[:, h:2 * h], AF.Exp,
                        scale=-TEMPERATURE)
            nc.sync.dma_start(out[ts(qt, 128), h:2 * h], st2[:, h:2 * h])
        else:
            chained_act(st2[:], st[:], AF.Exp, scale=-TEMPERATURE)
            nc.sync.dma_start(out[ts(qt, 128), 0:nch * 512], st2[:])

    def emit_qtile(qt, last=False, pool=None):
        cl0 = qt // 4
        nch = 2 * (4 - cl0)
        cols = [cl * 512 for cl in range(cl0, 4)] + \
               [NH + cl * 512 for cl in range(cl0, 4)]
        st = st_pool.tile([128, nch * 512], F16, tag="st", name=f"st{qt}")
        for s0 in range(0, nch, 2):
            seg = cols[s0:s0 + 2]
            ps = (pool or ps_pool).tile(
                [128, 1024], F32, tag="d2", name=f"d2_{qt}_{s0}")
            for j, col in enumerate(seg):
                psl = ps[:, ts(j, 512)]
                nc.tensor.matmul(
                    psl, hk[:, ts(qt, 128)], hk[:, col:col + 512],
                    start=True, stop=False,
                )
                nc.tensor.matmul(
                    psl, ones1r[:], augk[:, col:col + 512],
                    start=False, stop=True,
                )
            sw = len(seg) * 512
            sl = st[:, s0 * 512:s0 * 512 + sw]
            if qt in ACT_SQRT_QT:
                # fused drain+sqrt: sqrt(-2*psum + sq_q)
                chained_act(sl, ps[:, 0:sw], AF.Sqrt,
                            bias=sqq_cols[:, qt:qt + 1], scale=-2.0)
            else:
                # d2 = -2*psum + sq_q
                nc.vector.tensor_scalar(
                    sl, ps[:, 0:sw], -2.0, sqq_cols[:, qt:qt + 1],
                    ALU.mult, ALU.add,
                )
                if last or nch == 8:  # seg-granular sqrt: shorter chain
                    nc.gpsimd.tensor_tensor(
                        sl, sl, half16[:, 0:sw], ALU.pow
                    )
        if qt not in ACT_SQRT_QT and not last and nch != 8:
            nc.gpsimd.tensor_tensor(
                st[:], st[:], half16[:, 0:nch * 512], ALU.pow
            )
        pending.append((qt, st, nch, last))
        if qt in (15, 7):  # flush after each ACT sqrt phase closes
            for item in pending:
                emit_tail(*item)
            pending.clear()
        elif qt not in (10, 9, 8):  # hold so S8,S7 precede E10,E9,E8 in chain
            emit_tail(*pending.pop())

    # project chunks 3,7 first (the small qtiles need only those), then the
    # ACT-drained small qtiles stream while the remaining chunks project on
    # the warmed PE; after that the whole schedule is projection-free
    project_mm(3)
    project_mm(7)
    project_norms(3)
    project_norms(7)
    for qt in (15, 14, 13, 12):
        emit_qtile(qt)
    project_mm(2)
    project_mm(6)
    project_norms(2)
    project_norms(6)
    for qt in (11, 10, 9, 8):
        emit_qtile(qt)
    project_mm(1)
    project_mm(5)
    project_norms(1)
    project_norms(5)
    project_mm(0)
    project_mm(4)
    project_norms(0)
    project_norms(4)
    setup_ctx.close()
    ps_pool2 = ctx.enter_context(tc.tile_pool(name="d2b", bufs=2, space="PSUM"))
    # qt7 (the ACT-drained one) first in the tail group so its PSUM tiles
    # recycle early instead of queueing behind the exp chain; alternate psum
    # pools for double pipeline depth
    for i, qt in enumerate((7, 3, 2, 1, 0, 6, 5)):
        emit_qtile(qt, pool=ps_pool2 if i % 2 else None)
    emit_qtile(4, last=True, pool=ps_pool2)


def build_nc():
    nc = bacc.Bacc("TRN2", target_bir_lowering=False, debug=False)
    xt = nc.dram_tensor("xt", [D, N], F32, kind="ExternalInput").ap()
    wt = nc.dram_tensor("wt", [D, D], F32, kind="ExternalInput").ap()
    b = nc.dram_tensor("b", [D, 1], F32, kind="ExternalInput").ap()
    out = nc.dram_tensor("out", [NH, N], BF16, kind="ExternalOutput").ap()
    with tile.TileContext(nc) as tc:
        with ExitStack() as ctx:
            kernel_body(ctx, tc, out, xt, wt, b)
    nc.compile()
    return nc


_NC_CACHE = None


def _get_nc():
    global _NC_CACHE
    if _NC_CACHE is None:
        _NC_CACHE = build_nc()
    return _NC_CACHE


def _run(x, W, b, trace=False, **spmd_kwargs):
    from concourse.bass_utils import run_bass_kernel_spmd

    x = np.asarray(x, dtype=np.float32)
    W = np.asarray(W, dtype=np.float32)
    b = np.asarray(b, dtype=np.float32).reshape(D, 1)
    wt = np.ascontiguousarray(W.T)
    nc = _get_nc()
    in_maps = []
    for c in range(N_CORES):
        bi, qh = c // 2, c % 2
        xtb = x[bi].T  # [128, 4096]
        own = xtb[:, qh * NH:(qh + 1) * NH]
        oth = xtb[:, (1 - qh) * NH:(2 - qh) * NH]
        in_maps.append({
            "xt": np.ascontiguousarray(np.concatenate([own, oth], axis=1)),
            "wt": wt,
            "b": b,
        })
    res = run_bass_kernel_spmd(
        nc, in_maps, core_ids=list(range(N_CORES)), trace=trace, **spmd_kwargs
    )
    out = np.empty((B, N, N), dtype=np.float32)
    for c in range(N_CORES):
        bi, qh = c // 2, c % 2
        R = res.results[c]["out"].astype(np.float32)  # [2048, 4096] packed
        rows = slice(qh * NH, (qh + 1) * NH)
        M = out[bi]
        for qt in range(NQT):
            cl0 = qt // 4
            w = (4 - cl0) * 512
            r = slice(qh * NH + qt * 128, qh * NH + qt * 128 + 128)
            rr = slice(qt * 128, qt * 128 + 128)
            M[r, qh * NH + cl0 * 512:(qh + 1) * NH] = R[rr, 0:w]
            M[r, (1 - qh) * NH + cl0 * 512:(2 - qh) * NH] = R[rr, w:2 * w]
    # mirror the missing block-lower-triangle tiles of every 2048x2048
    # quadrant from their transposed counterparts, then pin the exact
    # diagonal to exp(-dist(m,m)) = 1 (this also clears the NaNs that
    # sqrt of the diagonal's tiny negative psum noise produces)
    for bi in range(B):
        M = out[bi]
        for r0 in (0, NH):
            for c0 in (0, NH):
                for qt in range(NQT):
                    r = slice(r0 + qt * 128, r0 + qt * 128 + 128)
                    for cl in range(qt // 4):
                        cs = slice(c0 + cl * 512, c0 + cl * 512 + 512)
                        M[r, cs] = M[cs, r].T
        np.fill_diagonal(M, 1.0)
    return out, res


def kernel(x, W, b):
    out, _ = _run(x, W, b)
    return out


# revision 42
# speedup vs baseline: 1.1719x; 1.0383x over previous
"""Self-contained TRN2 Bass kernel for nn_EuclideanSimilarity.

Full-input contract: kernel(x, W, b) with
  x [4, 4096, 128] f32, W [128, 128] f32, b [128] f32
returns out [4, 4096, 4096] f32 = exp(-pairwise_euclidean_dist(x @ W.T + b)).

Sharding + symmetry: 8 cores, core c -> (batch c//2, half c%2).  Each batch's
similarity matrix is SYMMETRIC, so a core computes only the chunk-level upper
triangle of its two 2048x2048 quadrants: self (own-half queries x own-half
keys) and cross (own-half queries x other-half keys).  Core 2b covers D1 and
B-upper; core 2b+1 covers D2 and (B^T)-upper = B-lower.  The host mirrors the
missing block-lower tiles from their (always directly computed) transposed
counterparts.  Each core's key tensor is ordered [own half | other half], so
all 8 cores run an IDENTICAL program (SPMD); only the host-side column
mapping differs.  Per-core work: 80 of 128 [128,512] output chunks.

Numerics: x^T and W^T are prepared on the host (input marshalling) so the
device does no input transposes.  h^T = W @ x^T + b is computed per 512-token
chunk in fp32 and rounded ONCE to fp32r; queries are the self-half keys, so 8
projection matmuls cover everything.  The gram matmul of fp32r operands is
exact (24-bit products, fp32 accumulate) and the norms are computed from the
SAME rounded values, so d2 = sq[m] + sq[n] - 2 h_m.h_n is the exact pairwise
distance of the rounded vectors - no catastrophic cancellation; h_r vs h is a
~2^-12 relative perturbation, invisible through exp(-sqrt).  The exact
diagonal (true d2 = 0, computed 0 +/- psum accumulation noise, possibly NaN
after sqrt of a tiny negative) is pinned to exp(0) = 1 on the host during
unsharding, which also clears those NaNs.

Per [128,512] chunk the PE does TWO fp32r matmuls: gram (h queries x keys,
K=128) and a K=1 "aug" adding -sq_k[n]/2 (ones lhsT x norm-row rhs).  The
drain then computes d2 = -2*(psum) + sq_q[m]: for ACT-assigned qtiles as one
fused ACT op sqrt(-2*psum + sq_q) straight from PSUM (sqrt table), otherwise
as a DVE tensor_scalar into fp16 followed by sqrt as pow(x, 0.5) on the
GPSIMD (a third elementwise lane).  exp(-x) runs on ACT into bf16 (one exp
table load), leaves as bf16 (half DMA bytes, one packed DMA per qtile), and
is upcast on the host during unsharding.  Qtiles are processed 15->0 (small
ones first - they need only the first-loaded key chunks, so the main loop
starts ~5us in, overlapping the rest of the projection prologue).
"""

from contextlib import ExitStack

import numpy as np

import concourse.mybir as mybir
import concourse.tile as tile
from concourse.tile import add_dep_helper
from concourse import bacc
from concourse.bass import ts
from concourse.masks import make_identity

F32 = mybir.dt.float32
F32R = mybir.dt.float32r
F16 = mybir.dt.float16
BF16 = mybir.dt.bfloat16
AF = mybir.ActivationFunctionType
ALU = mybir.AluOpType

B = 4
N = 4096
NH = 2048   # tokens per half (queries per core)
D = 128
TEMPERATURE = 1.0
NQT = NH // 128  # 16 query tiles per core
N_CORES = 8

# qtiles whose drain+sqrt runs fused on ACT (sqrt table); the rest drain via
# DVE and take sqrt on the GPSIMD pow lane.  The small qtiles run first (they
# need only the first-loaded key chunks, overlapping the prologue), and one
# mid-size ACT qtile sits in the final group to balance the engines.
ACT_SQRT_QT = (15, 8, 7, 6)


def kernel_body(ctx: ExitStack, tc: tile.TileContext, out, xt, wt, b):
    nc = tc.nc

    consts = ctx.enter_context(tc.tile_pool(name="consts", bufs=1))
    # preload the sqrt table set while the prologue runs
    scrap = consts.tile([1, 8], F32)
    nc.gpsimd.memset(scrap[:], 1.0)
    nc.scalar.activation(scrap[:], scrap[:], AF.Sqrt)

    ident = consts.tile([128, 128], F32)
    make_identity(nc, ident[:])

    wt_sb = consts.tile([128, 128], F32)
    nc.sync.dma_start(wt_sb[:], wt[:, :])
    b_sb = consts.tile([128, 1], F32)
    nc.sync.dma_start(b_sb[:], b[:, :])
    ones_col = consts.tile([128, 1], F32)    # lhsT for the f32 sq matmul
    nc.gpsimd.memset(ones_col[:], 1.0)
    ones_row = consts.tile([1, 128], F32)
    nc.gpsimd.memset(ones_row[:], 1.0)
    ones1r = consts.tile([1, 128], F32R)     # K=1 lhsT for the aug matmul
    nc.vector.tensor_copy(ones1r[:], ones_row[:])
    half16 = consts.tile([128, N], F16)      # pow exponent for gpsimd sqrt
    nc.vector.memset(half16[:], 0.5)

    # persistent operands
    h_pool = ctx.enter_context(tc.tile_pool(name="h", bufs=1))
    hk = h_pool.tile([128, N], F32R)         # h for all 4096 keys
    augk = h_pool.tile([1, N], F32R)         # -sq_k/2 row
    sqq_cols = h_pool.tile([128, NQT], F32)  # sq_q column per qtile
    sq_row = h_pool.tile([1, NH], F32)       # self-half norms (fp32)

    # main-loop pools created first so the (scoped) projection pools sit on
    # top of the pool stack and can be released before the tail group
    st_pool = ctx.enter_context(tc.tile_pool(name="st", bufs=7))
    st2_pool = ctx.enter_context(tc.tile_pool(name="st2", bufs=7))
    ps_pool = ctx.enter_context(tc.tile_pool(name="d2", bufs=2, space="PSUM"))

    # projection pools (coexist with the early main loop; 4 PSUM banks,
    # closed after the last chunk so the tail group gets a second psum pool)
    setup_ctx = ExitStack()
    ssb = setup_ctx.enter_context(tc.tile_pool(name="setup_sb", bufs=3))
    sps = setup_ctx.enter_context(tc.tile_pool(name="setup_ps", bufs=1, space="PSUM"))

    # PE p-state warmup: ~60 tiny matmuls keep the tensor engine busy from
    # t~1us so its clock is fully ramped (3us of continuous execution) by the
    # time the first real fp32 projection matmul dispatches — those would
    # otherwise run 2-3x slower and serialize the whole prologue.
    warm = sps.tile([128, 16], F32, tag="sqqtp", bufs=1, name="warm")
    for i in range(60):
        nc.tensor.matmul(warm[:], ident[:], ident[:, 0:16],
                         start=True, stop=True)

    # prefetch all key chunks up front (the per-chunk DMA latency otherwise
    # serializes into the projection chain)
    xins = {}
    for c in (3, 7, 2, 6, 1, 5, 0, 4):
        xin = ssb.tile([128, 512], F32, tag="xin", bufs=8, name=f"xin{c}")
        nc.sync.dma_start(xin[:], xt[:, ts(c, 512)])
        xins[c] = xin

    s2fs = {}

    def project_mm(c):
        hps = sps.tile([128, 512], F32, tag="hps", bufs=2, name=f"hps{c}")
        nc.tensor.matmul(hps[:], wt_sb[:], xins[c][:], start=True, stop=True)
        nc.vector.tensor_scalar_add(hk[:, ts(c, 512)], hps[:], b_sb[:, 0:1])
        s2f = ssb.tile([128, 512], F32, tag="s2f", bufs=8, name=f"s2f{c}")
        nc.vector.tensor_mul(s2f[:], hk[:, ts(c, 512)], hk[:, ts(c, 512)])
        s2fs[c] = s2f

    def project_norms(c):
        sqps = sps.tile([128, 512], F32, tag="sqps", bufs=1, name=f"sqps{c}")
        nc.tensor.matmul(
            sqps[0:1, :], ones_col[:], s2fs.pop(c)[:], start=True, stop=True
        )
        nc.scalar.activation(
            augk[0:1, ts(c, 512)], sqps[0:1, :], AF.Identity, scale=-0.5
        )
        if c < NH // 512:  # self chunk: also query norms (exact fp32)
            nc.vector.tensor_copy(sq_row[0:1, ts(c, 512)], sqps[0:1, :])
            for j in range(4):
                qt = 4 * c + j
                nc.tensor.transpose(
                    warm[:, qt % 4:qt % 4 + 1], sq_row[0:1, ts(qt, 128)],
                    ident[0:1, 0:1],
                )
            nc.vector.tensor_copy(
                sqq_cols[:, 4 * c:4 * c + 4], warm[:, 0:4]
            )

    def project_chunk(c):
        project_mm(c)
        project_norms(c)

    # ---------------- main loop ----------------
    last_act = [None]

    def chained_act(*args, **kwargs):
        bi = nc.scalar.activation(*args, **kwargs)
        if last_act[0] is not None:
            add_dep_helper(bi.ins, last_act[0].ins, sync=False,
                           reason="act-table-order")
        last_act[0] = bi
        return bi

    pending = []

    def emit_tail(qt, st, nch, split):
        st2 = st2_pool.tile([128, nch * 512], BF16, tag="st2", name=f"st2_{qt}")
        # NaNs from sqrt of tiny diagonal negatives pass through; the host
        # pins the exact diagonal to exp(0)=1 afterwards.
        if split:  # final qtile: halve exp+DMA so the last DMA overlaps
            h = nch * 256
            chained_act(st2[:, 0:h], st[:, 0:h], AF.Exp, scale=-TEMPERATURE)
            nc.sync.dma_start(out[ts(qt, 128), 0:h], st2[:, 0:h])
            chained_act(st2[:, h:2 * h], st[:, h:2 * h], AF.Exp,
                        scale=-TEMPERATURE)
            nc.sync.dma_start(out[ts(qt, 128), h:2 * h], st2[:, h:2 * h])
        else:
            chained_act(st2[:], st[:], AF.Exp, scale=-TEMPERATURE)
            nc.sync.dma_start(out[ts(qt, 128), 0:nch * 512], st2[:])

    def emit_qtile(qt, last=False, pool=None):
        cl0 = qt // 4
        nch = 2 * (4 - cl0)
        cols = [cl * 512 for cl in range(cl0, 4)] + \
               [NH + cl * 512 for cl in range(cl0, 4)]
        st = st_pool.tile([128, nch * 512], F16, tag="st", name=f"st{qt}")
        for s0 in range(0, nch, 2):
            seg = cols[s0:s0 + 2]
            ps = (pool or ps_pool).tile(
                [128, 1024], F32, tag="d2", name=f"d2_{qt}_{s0}")
            for j, col in enumerate(seg):
                psl = ps[:, ts(j, 512)]
                nc.tensor.matmul(
                    psl, hk[:, ts(qt, 128)], hk[:, col:col + 512],
                    start=True, stop=False,
                )
                nc.tensor.matmul(
                    psl, ones1r[:], augk[:, col:col + 512],
                    start=False, stop=True,
                )
            sw = len(seg) * 512
            sl = st[:, s0 * 512:s0 * 512 + sw]
            if qt in ACT_SQRT_QT:
                # fused drain+sqrt: sqrt(-2*psum + sq_q)
                chained_act(sl, ps[:, 0:sw], AF.Sqrt,
                            bias=sqq_cols[:, qt:qt + 1], scale=-2.0)
            else:
                # d2 = -2*psum + sq_q
                nc.vector.tensor_scalar(
                    sl, ps[:, 0:sw], -2.0, sqq_cols[:, qt:qt + 1],
                    ALU.mult, ALU.add,
                )
                if last or nch == 8:  # seg-granular sqrt: shorter chain
                    nc.gpsimd.tensor_tensor(
                        sl, sl, half16[:, 0:sw], ALU.pow
                    )
        if qt not in ACT_SQRT_QT and not last and nch != 8:
            nc.gpsimd.tensor_tensor(
                st[:], st[:], half16[:, 0:nch * 512], ALU.pow
            )
        pending.append((qt, st, nch, last))
        if qt in (15, 6):  # flush after each ACT sqrt phase closes
            for item in pending:
                emit_tail(*item)
            pending.clear()
        elif qt not in (10, 9, 8, 7):  # hold so S-phases precede held exps
            emit_tail(*pending.pop())

    # project chunks 3,7 first (the small qtiles need only those), then the
    # ACT-drained small qtiles stream while the remaining chunks project on
    # the warmed PE; after that the whole schedule is projection-free
    project_mm(3)
    project_mm(7)
    project_norms(3)
    project_norms(7)
    for qt in (15, 14, 13, 12):
        emit_qtile(qt)
    project_mm(2)
    project_mm(6)
    project_norms(2)
    project_norms(6)
    for qt in (11, 10, 9, 8):
        emit_qtile(qt)
    project_mm(1)
    project_mm(5)
    project_norms(1)
    project_norms(5)
    project_mm(0)
    project_mm(4)
    project_norms(0)
    project_norms(4)
    setup_ctx.close()
    ps_pool2 = ctx.enter_context(tc.tile_pool(name="d2b", bufs=2, space="PSUM"))
    # qt7 (the ACT-drained one) first in the tail group so its PSUM tiles
    # recycle early instead of queueing behind the exp chain; alternate psum
    # pools for double pipeline depth
    for i, qt in enumerate((7, 6, 3, 2, 1, 0, 5)):
        emit_qtile(qt, pool=ps_pool2 if i % 2 else None)
    emit_qtile(4, last=True, pool=ps_pool2)


def build_nc():
    nc = bacc.Bacc("TRN2", target_bir_lowering=False, debug=False)
    xt = nc.dram_tensor("xt", [D, N], F32, kind="ExternalInput").ap()
    wt = nc.dram_tensor("wt", [D, D], F32, kind="ExternalInput").ap()
    b = nc.dram_tensor("b", [D, 1], F32, kind="ExternalInput").ap()
    out = nc.dram_tensor("out", [NH, N], BF16, kind="ExternalOutput").ap()
    with tile.TileContext(nc) as tc:
        with ExitStack() as ctx:
            kernel_body(ctx, tc, out, xt, wt, b)
    nc.compile()
    return nc


_NC_CACHE = None


def _get_nc():
    global _NC_CACHE
    if _NC_CACHE is None:
        _NC_CACHE = build_nc()
    return _NC_CACHE


def _run(x, W, b, trace=False, **spmd_kwargs):
    from concourse.bass_utils import run_bass_kernel_spmd

    x = np.asarray(x, dtype=np.float32)
    W = np.asarray(W, dtype=np.float32)
    b = np.asarray(b, dtype=np.float32).reshape(D, 1)
    wt = np.ascontiguousarray(W.T)
    nc = _get_nc()
    in_maps = []
    for c in range(N_CORES):
        bi, qh = c // 2, c % 2
        xtb = x[bi].T  # [128, 4096]
        own = xtb[:, qh * NH:(qh + 1) * NH]
        oth = xtb[:, (1 - qh) * NH:(2 - qh) * NH]
        in_maps.append({
            "xt": np.ascontiguousarray(np.concatenate([own, oth], axis=1)),
            "wt": wt,
            "b": b,
        })
    res = run_bass_kernel_spmd(
        nc, in_maps, core_ids=list(range(N_CORES)), trace=trace, **spmd_kwargs
    )
    out = np.empty((B, N, N), dtype=np.float32)
    for c in range(N_CORES):
        bi, qh = c // 2, c % 2
        R = res.results[c]["out"].astype(np.float32)  # [2048, 4096] packed
        rows = slice(qh * NH, (qh + 1) * NH)
        M = out[bi]
        for qt in range(NQT):
            cl0 = qt // 4
            w = (4 - cl0) * 512
            r = slice(qh * NH + qt * 128, qh * NH + qt * 128 + 128)
            rr = slice(qt * 128, qt * 128 + 128)
            M[r, qh * NH + cl0 * 512:(qh + 1) * NH] = R[rr, 0:w]
            M[r, (1 - qh) * NH + cl0 * 512:(2 - qh) * NH] = R[rr, w:2 * w]
    # mirror the missing block-lower-triangle tiles of every 2048x2048
    # quadrant from their transposed counterparts, then pin the exact
    # diagonal to exp(-dist(m,m)) = 1 (this also clears the NaNs that
    # sqrt of the diagonal's tiny negative psum noise produces)
    for bi in range(B):
        M = out[bi]
        for r0 in (0, NH):
            for c0 in (0, NH):
                for qt in range(NQT):
                    r = slice(r0 + qt * 128, r0 + qt * 128 + 128)
                    for cl in range(qt // 4):
                        cs = slice(c0 + cl * 512, c0 + cl * 512 + 512)
                        M[r, cs] = M[cs, r].T
        np.fill_diagonal(M, 1.0)
    return out, res


def kernel(x, W, b):
    out, _ = _run(x, W, b)
    return out
